# revision 9
# baseline (speedup 1.0000x reference)
"""Trainium2 Bass kernel for the BotImpact GNN (2x GATConv + heads), 8 NeuronCores.

Strategy (dst-sharded graph parallel):
- Nodes are assigned to 8 cores by id (6250/core); within a core, nodes are
  permuted by a 2-level in-degree sort so fixed-size edge-slot columns pad tightly.
- Each core builds its shard of a "table" [h(bf16) | alpha_src(f32)] rows (512B)
  for each GAT layer, AllGathers the full table, then aggregates its own
  destination nodes: for each dst group of 128, per-edge-slot columns are
  fetched with dma_gather (4 SWDGE queues), scored
  (ex = max(exp(s), exp(0.2 s)) == exp(leaky_relu(s, 0.2))), and accumulated
  agg += ex * h with per-partition FMAs. Softmax normalization commutes with
  the weighted sum, so denominators divide once per group at the end
  (no segment-max needed: scores are O(1)-bounded).
- int16 gather indices cover only 32768 rows, so columns are split into
  A-half / B-half streams with different base offsets; padding slots point at
  sentinel rows whose alpha_src = -1e4 makes their weight exactly 0.
- Output heads (treat/control MLPs, tprob) run on-device; host only
  permutes/scatters results back.
"""
import os
import sys
import numpy as np

P = 128
NCORES = 8
HEADS = 2

_PROGRAM_CACHE = {}


# ----------------------------------------------------------------------------
# Host-side preprocessing
# ----------------------------------------------------------------------------

def _positions_for_core(a0, b0, local):
    """2-level sort: descending band-sort by a0 (band 512), then by b0."""
    o1 = np.argsort(-a0, kind="stable")
    for s in range(0, local, 512):
        band = o1[s:s + 512]
        o1[s:s + 512] = band[np.argsort(-b0[band], kind="stable")]
    pos = np.empty(local, np.int64)
    pos[o1] = np.arange(local)
    return pos


def prep_graph(edge_index, N, local, groups, shard, half_rows):
    """Returns (row_of_node [N], per-core idx blocks + uniform instr metadata)."""
    ncg = N // local
    assert ncg == NCORES
    src = np.concatenate([edge_index[0].astype(np.int64), np.arange(N)])
    dst = np.concatenate([edge_index[1].astype(np.int64), np.arange(N)])

    # approximate A-membership for the sort key (exact afterwards)
    acore = (half_rows // shard)  # cores fully inside half A
    srcA0 = (src // local) < acore
    a0 = np.bincount(dst[srcA0], minlength=N).astype(np.int64)
    b0 = np.bincount(dst[~srcA0], minlength=N).astype(np.int64)

    pos = np.empty(N, np.int64)
    for c in range(ncg):
        sl = slice(c * local, (c + 1) * local)
        pos[sl] = _positions_for_core(a0[sl], b0[sl], local)
    row_of = (np.arange(N) // local) * shard + pos

    srow = row_of[src]
    dcore = dst // local
    dpos = pos[dst]
    isB = srow >= half_rows

    # rank of each edge within its (dst, half) bucket
    key = (dcore * local + dpos) * 2 + isB
    order = np.lexsort((np.arange(len(key)), key))
    ks = key[order]
    first = np.zeros(len(ks), np.int64)
    newk = np.ones(len(ks), bool)
    newk[1:] = ks[1:] != ks[:-1]
    starts = np.flatnonzero(newk)
    cnt = np.diff(np.append(starts, len(ks)))
    rank_sorted = np.arange(len(ks)) - np.repeat(starts, cnt)
    rank = np.empty(len(ks), np.int64)
    rank[order] = rank_sorted

    # per (core, pos) A/B counts
    flat = dcore * local + dpos
    na = np.bincount(flat[~isB], minlength=ncg * local).reshape(ncg, local)
    nb = np.bincount(flat[isB], minlength=ncg * local).reshape(ncg, local)

    # per-group K (uniform across cores)
    npos = groups * P  # padded position space
    na_p = np.zeros((ncg, npos), np.int64)
    nb_p = np.zeros((ncg, npos), np.int64)
    na_p[:, :local] = na
    nb_p[:, :local] = nb
    KA = na_p.reshape(ncg, groups, P).max(axis=(0, 2))
    KB = nb_p.reshape(ncg, groups, P).max(axis=(0, 2))

    sentA = local if local % P == 0 else groups * P  # sentinel row local id
    sent_local = groups * P  # sentinel group base (local)
    SENT_A = sent_local      # core 0 sentinel row (< half_rows)
    # B sentinel: first core whose shard starts >= half_rows
    bcore = (half_rows + shard - 1) // shard
    SENT_B = bcore * shard + sent_local - half_rows
    assert 0 <= SENT_B < 32768

    # column value arrays [core, group, k, p]
    colA = np.full((ncg, groups, max(int(KA.max()), 1), P), SENT_A, np.int64)
    colB = np.full((ncg, groups, max(int(KB.max()), 1), P), SENT_B, np.int64)
    selA = ~isB
    colA[dcore[selA], dpos[selA] // P, rank[selA], dpos[selA] % P] = srow[selA]
    selB = isB
    colB[dcore[selB], dpos[selB] // P, rank[selB], dpos[selB] % P] = \
        srow[selB] - half_rows

    # instruction stream metadata (uniform): list of (half, [(g,k),...<=8]).
    # Interleave A/B buffers group-by-group so each group's accumulator tile
    # has a short lifetime in the schedule.
    instrs = []
    bufA, bufB = [], []
    for g in range(groups):
        for k in range(int(KA[g])):
            bufA.append((g, k))
            if len(bufA) == 8:
                instrs.append(("A", bufA))
                bufA = []
        for k in range(int(KB[g])):
            bufB.append((g, k))
            if len(bufB) == 8:
                instrs.append(("B", bufB))
                bufB = []
    if bufA:
        instrs.append(("A", bufA))
    if bufB:
        instrs.append(("B", bufB))

    # idx tensors per core: [P, n_instr*64] int16
    n_instr = len(instrs)
    idx = np.full((ncg, P, n_instr * 64), -1, np.int16)
    for i, (half, cols) in enumerate(instrs):
        blk = np.full((ncg, 1024), -1, np.int64)
        arr = colA if half == "A" else colB
        for j, (g, k) in enumerate(cols):
            blk[:, j * P:(j + 1) * P] = arr[:, g, k, :]
        tmp = blk.reshape(ncg, 64, 16).transpose(0, 2, 1)  # [ncg, 16, 64]
        idx[:, :, i * 64:(i + 1) * 64] = np.tile(tmp, (1, 8, 1)).astype(np.int16)
    return row_of, idx, instrs


def _head_lists(node_idx, row_of, local, shard):
    """Per-core local rows + (core, slot) mapping for output scatter."""
    node_idx = node_idx.astype(np.int64)
    core = node_idx // local
    lrow = row_of[node_idx] - core * shard
    counts = np.bincount(core, minlength=NCORES)
    cap = int(np.ceil(max(int(counts.max()), 1) / P) * P)
    rows = np.zeros((NCORES, cap), np.int64)
    slot = np.zeros(len(node_idx), np.int64)
    fill = np.zeros(NCORES, np.int64)
    for j in range(len(node_idx)):
        c = core[j]
        rows[c, fill[c]] = lrow[j]
        slot[j] = fill[c]
        fill[c] += 1
    return rows, core, slot, cap


def _idx16(rows_blk):
    """[cap] local rows -> [P, cap//16] int16 gather-idx layout (per core)."""
    cap = rows_blk.shape[-1]
    tmp = rows_blk.reshape(-1, cap // 16, 16)         # [B, cap/16, 16]
    tmp = np.swapaxes(tmp, -1, -2)                    # [B, 16, cap/16]
    return np.tile(tmp, (1, 8, 1)).astype(np.int16)   # [B, 128, cap/16]


def _fold_w(W, a_src, a_dst):
    """Wext [IN, HD+4] = [W | W@Asrc | W@Adst]."""
    hd = W.shape[1]
    hdim = a_src.shape[1]
    A_s = np.zeros((hd, HEADS), np.float32)
    A_d = np.zeros((hd, HEADS), np.float32)
    for h in range(HEADS):
        A_s[h * hdim:(h + 1) * hdim, h] = a_src[h]
        A_d[h * hdim:(h + 1) * hdim, h] = a_dst[h]
    return np.concatenate([W, W @ A_s, W @ A_d], axis=1).astype(np.float32)


# ----------------------------------------------------------------------------
# Device program
# ----------------------------------------------------------------------------

def build_program(cfg):
    sys.path.insert(0, "/opt/trn_rl_repo")
    import concourse.bass as bass
    import concourse.bacc as bacc
    import concourse.mybir as mybir
    from concourse.tile import TileContext
    from concourse.masks import make_identity

    groups = cfg["groups"]
    shard = cfg["shard"]
    half_rows = cfg["half_rows"]
    tab_rows = shard * NCORES
    in_dim = cfg["in_dim"]
    hd = cfg["hd"]          # 128
    hdim = hd // HEADS      # 64
    rw = hd * 2             # bf16 elems per row (512B)
    instrs_r = cfg["instrs_r"]
    instrs_f = cfg["instrs_f"]
    tcap = cfg["tcap"]
    f32 = mybir.dt.float32
    bf16 = mybir.dt.bfloat16
    i16 = mybir.dt.int16
    AL = mybir.AluOpType
    ACT = mybir.ActivationFunctionType

    nc = bacc.Bacc("TRN2", target_bir_lowering=False, debug=False,
                   num_devices=NCORES, num_swdge_queues=4)

    # ---- I/O ----
    def din(name, shape, dt=f32):
        return nc.dram_tensor(name, shape, dt, kind="ExternalInput")

    def dout(name, shape, dt=f32):
        return nc.dram_tensor(name, shape, dt, kind="ExternalOutput")

    xT = {g: din(f"xT_{g}", [P, groups * P]) for g in "rf"}
    nidx = {g: din(f"idx_{g}", [P, len(cfg[f"instrs_{g}"]) * 64], i16) for g in "rf"}
    Wext1 = din("Wext1", [in_dim, hd + 4])
    Wext2 = din("Wext2", [hd, hd + 4])
    b1b = din("b1b", [P, hd])
    b2b = din("b2b", [P, hd])
    WyS = din("WyS", [hd, hdim])
    bySb = din("bySb", [P, hdim])
    Wy1 = din("Wy1", [hdim, 1])
    Wy0 = din("Wy0", [hdim, 1])
    by1b = din("by1b", [P, 1])
    by0b = din("by0b", [P, 1])
    Wp = din("Wp", [hd, 2])
    bpb = din("bpb", [P, 2])
    hidx = {k: din(f"hidx_{k}", [P, tcap // 16], i16)
            for k in ("tr", "cr", "tf", "cf")}

    xz2_out = dout("xz2", [shard, hd])
    xfz2_out = dout("xfz2", [shard, hd])
    tp_out = dout("tprob", [groups * P, 2])
    y_out = {k: dout(f"y_{k}", [P, tcap // P]) for k in ("tr", "cr", "tf", "cf")}
    if cfg.get("debug"):
        xz1_out = dout("xz1", [groups * P, hd])

    with TileContext(nc) as tc:
        with tc.tile_pool(name="const", bufs=1) as cpool, \
             tc.tile_pool(name="resident", bufs=1) as rpool, \
             tc.tile_pool(name="work", bufs=3) as wpool, \
             tc.tile_pool(name="gat", bufs=10) as gpool, \
             tc.tile_pool(name="agg", bufs=8) as apool, \
             tc.tile_pool(name="small", bufs=8) as spool, \
             tc.tile_pool(name="psum", bufs=3, space="PSUM") as psum, \
             tc.tile_pool(name="dram", bufs=1, space="DRAM") as dram:

            ident = cpool.tile([P, P], f32)
            make_identity(nc, ident[:])
            we1 = cpool.tile([in_dim, hd + 4], f32)
            nc.sync.dma_start(out=we1[:], in_=Wext1[:])
            we2 = cpool.tile([hd, hd + 4], f32)
            nc.sync.dma_start(out=we2[:], in_=Wext2[:])
            b1t = cpool.tile([P, hd], f32)
            nc.sync.dma_start(out=b1t[:], in_=b1b[:])
            b2t = cpool.tile([P, hd], f32)
            nc.sync.dma_start(out=b2t[:], in_=b2b[:])

            # DRAM internals
            tab_own = {}
            tab_all = {}
            for g in "rf":
                for l in (1, 2):
                    tab_own[g, l] = dram.tile([shard, rw], bf16, name=f"tabown_{g}{l}")
                    tab_all[g, l] = dram.tile([tab_rows, rw], bf16,
                                              addr_space="Shared",
                                              name=f"taball_{g}{l}")

            # resident tiles
            ad = {}      # alpha_dst per (graph, layer): [P, groups, 2]
            xz1d = {}    # layer-1 activations per graph, spilled to DRAM
            for g in "rf":
                for l in (1, 2):
                    ad[g, l] = rpool.tile([P, groups, 2], f32, tag=f"ad{g}{l}", name=f"ad_{g}{l}")
                xz1d[g] = dram.tile([groups * P, hd], f32, name=f"xz1d_{g}")
            idx_t = {}
            for g in "rf":
                idx_t[g] = rpool.tile([P, len(cfg[f"instrs_{g}"]) * 64], i16,
                                      tag=f"idx{g}", name=f"idxt_{g}")
                nc.sync.dma_start(out=idx_t[g][:], in_=nidx[g][:])

            def build_table(g, layer):
                """Build tab_own[g, layer] (+ resident ad), group by group."""
                wt = we1 if layer == 1 else we2
                if layer == 1:
                    xsrc = rpool.tile([P, groups * P], f32, tag="xTs")
                    nc.sync.dma_start(out=xsrc[:], in_=xT[g][:])
                for gi in range(groups):
                    if layer == 1:
                        lhs = xsrc[:, gi * P:(gi + 1) * P]
                    else:
                        xzl = wpool.tile([P, hd], f32, tag="xzl")
                        nc.sync.dma_start(
                            out=xzl[:], in_=xz1d[g][gi * P:(gi + 1) * P, :])
                        tp = psum.tile([P, P], f32, space="PSUM", tag="tp", name="tp")
                        nc.tensor.transpose(out=tp[:], in_=xzl[:],
                                            identity=ident[:])
                        tsb = wpool.tile([P, P], f32, tag="tsb")
                        nc.vector.tensor_copy(out=tsb[:], in_=tp[:])
                        lhs = tsb[:]
                    hp = psum.tile([P, hd + 4], f32, space="PSUM", tag="mm", name="hp")
                    nc.tensor.matmul(hp[:], lhsT=lhs, rhs=wt[:],
                                     start=True, stop=True)
                    rowt = wpool.tile([P, rw], bf16, tag="rowt")
                    nc.vector.tensor_copy(out=rowt[:, :hd], in_=hp[:, :hd])
                    rf32 = rowt[:].bitcast(f32)
                    nc.vector.memset(rf32[:, hdim + 2:], 0.0)
                    nc.vector.tensor_copy(out=rf32[:, hdim:hdim + 2],
                                          in_=hp[:, hd:hd + 2])
                    nc.vector.tensor_copy(out=ad[g, layer][:, gi, :],
                                          in_=hp[:, hd + 2:hd + 4])
                    nc.sync.dma_start(
                        out=tab_own[g, layer][gi * P:(gi + 1) * P, :],
                        in_=rowt[:])
                # sentinel group
                sent = wpool.tile([P, rw], bf16, tag="rowt")
                nc.vector.memset(sent[:], 0.0)
                nc.vector.memset(sent[:].bitcast(f32)[:, hdim:hdim + 2], -1e4)
                nc.sync.dma_start(
                    out=tab_own[g, layer][groups * P:groups * P + P, :],
                    in_=sent[:])

            def allgather(g, layer):
                nc.gpsimd.collective_compute(
                    "AllGather", mybir.AluOpType.bypass,
                    replica_groups=[list(range(NCORES))],
                    ins=[tab_own[g, layer][:]],
                    outs=[tab_all[g, layer][:]])

            swdge_ord = [0]

            def next_q():
                q = (swdge_ord[0] % 8) % 4
                swdge_ord[0] += 1
                return q

            def conv(g, layer):
                """Gather-aggregate phase. Returns list of finalized [P,hd] tiles."""
                instrs = instrs_r if g == "r" else instrs_f
                table = tab_all[g, layer]
                adt = ad[g, layer]
                agg = {}
                den = {}
                ncols_left = {}
                for gi in range(groups):
                    ncols_left[gi] = 0
                for half, cols in instrs:
                    for (gi, k) in cols:
                        ncols_left[gi] += 1
                outs = [None] * groups
                for ii, (half, cols) in enumerate(instrs):
                    ncol = len(cols)
                    g_t = gpool.tile([P, 8, rw], bf16, tag="g")
                    base = table[:half_rows, :] if half == "A" \
                        else table[half_rows:, :]
                    nc.gpsimd.dma_gather(
                        g_t[:], base, idx_t[g][:, ii * 64:(ii + 1) * 64],
                        1024, ncol * P, rw, queue_num=next_q())
                    gf = g_t[:].bitcast(f32)  # [P, 8, hd]
                    ex = spool.tile([P, 8, 2], f32, tag="ex")
                    e2 = spool.tile([P, 8, 2], f32, tag="e2")
                    # per group-run: score add into ex
                    j = 0
                    while j < ncol:
                        gi = cols[j][0]
                        j2 = j
                        while j2 < ncol and cols[j2][0] == gi:
                            j2 += 1
                        if agg.get(gi) is None:
                            agg[gi] = apool.tile([P, hd], f32, tag="agg", name=f"agg_{gi}")
                            den[gi] = spool.tile([P, 2], f32, tag="den", name=f"den_{gi}")
                            nc.vector.memset(agg[gi][:], 0.0)
                            nc.vector.memset(den[gi][:], 0.0)
                        nc.vector.tensor_tensor(
                            out=ex[:, j:j2, :], in0=gf[:, j:j2, hdim:hdim + 2],
                            in1=adt[:, gi:gi + 1, :].to_broadcast([P, j2 - j, 2]),
                            op=AL.add)
                        j = j2
                    # exp(lrelu) = max(exp(x), exp(0.2x))
                    nc.scalar.activation(e2[:, :ncol, :], ex[:, :ncol, :],
                                         ACT.Exp, scale=0.2)
                    nc.scalar.activation(ex[:, :ncol, :], ex[:, :ncol, :],
                                         ACT.Exp)
                    nc.vector.tensor_tensor(out=ex[:, :ncol, :],
                                            in0=ex[:, :ncol, :],
                                            in1=e2[:, :ncol, :], op=AL.max)
                    # denom + FMA per run/column
                    j = 0
                    while j < ncol:
                        gi = cols[j][0]
                        j2 = j
                        while j2 < ncol and cols[j2][0] == gi:
                            j2 += 1
                        exv = ex[:, j:j2, :]
                        if j2 - j == 1:
                            nc.vector.tensor_tensor(out=den[gi][:],
                                                    in0=den[gi][:],
                                                    in1=ex[:, j, :], op=AL.add)
                        else:
                            red = spool.tile([P, 2], f32, tag="red")
                            ex_sw = bass.AP(exv.tensor, exv.offset,
                                            [exv.ap[0], exv.ap[2], exv.ap[1]])
                            nc.vector.tensor_reduce(out=red[:], in_=ex_sw,
                                                    axis=mybir.AxisListType.X,
                                                    op=AL.add)
                            nc.vector.tensor_tensor(out=den[gi][:],
                                                    in0=den[gi][:], in1=red[:],
                                                    op=AL.add)
                        for c in range(j, j2):
                            for h in range(HEADS):
                                sl = slice(h * hdim, (h + 1) * hdim)
                                nc.vector.scalar_tensor_tensor(
                                    out=agg[gi][:, sl],
                                    in0=g_t[:, c, sl], scalar=ex[:, c, h:h + 1],
                                    in1=agg[gi][:, sl],
                                    op0=AL.mult, op1=AL.add)
                        for c in range(j, j2):
                            ncols_left[gi] -= 1
                        if ncols_left[gi] == 0:
                            outs[gi] = finalize(g, layer, gi, agg[gi], den[gi])
                            agg[gi] = None
                        j = j2
                return outs

            def finalize(g, layer, gi, aggt, dent):
                bt = b1t if layer == 1 else b2t
                nc.vector.tensor_scalar_max(out=dent[:], in0=dent[:],
                                            scalar1=1e-30)
                dinv = spool.tile([P, 2], f32, tag="dinv")
                nc.vector.reciprocal(out=dinv[:], in_=dent[:])
                ot_t = apool.tile([P, hd], f32, tag="oc")
                ot = ot_t[:]
                for h in range(HEADS):
                    sl = slice(h * hdim, (h + 1) * hdim)
                    nc.vector.scalar_tensor_tensor(
                        out=ot[:, sl], in0=aggt[:, sl],
                        scalar=dinv[:, h:h + 1], in1=bt[:, sl],
                        op0=AL.mult, op1=AL.add)
                if layer == 1:
                    nc.vector.tensor_scalar_max(out=ot, in0=ot, scalar1=0.0)
                    nc.sync.dma_start(
                        out=xz1d[g][gi * P:(gi + 1) * P, :], in_=ot)
                    if cfg.get("debug") and g == "r":
                        nc.sync.dma_start(
                            out=xz1_out[gi * P:(gi + 1) * P, :], in_=ot)
                    return None
                # layer 2: store shard output
                dst = xz2_out if g == "r" else xfz2_out
                nc.sync.dma_start(out=dst[gi * P:(gi + 1) * P, :], in_=ot)
                if g == "r":
                    tprob(gi, ot)
                return ot_t

            def lrelu001(ap, tmp_tag):
                t = spool.tile(list(ap.shape), f32, tag=tmp_tag, name=tmp_tag)
                nc.vector.tensor_scalar_mul(out=t[:], in0=ap, scalar1=0.01)
                nc.vector.tensor_tensor(out=ap, in0=ap, in1=t[:], op=AL.max)

            def tprob(gi, ot):
                tpp = psum.tile([P, P], f32, space="PSUM", tag="tp", name="tpp")
                nc.tensor.transpose(out=tpp[:], in_=ot, identity=ident[:])
                tsb = wpool.tile([P, P], f32, tag="tsb")
                nc.vector.tensor_copy(out=tsb[:], in_=tpp[:])
                pp = psum.tile([P, 2], f32, space="PSUM", tag="mm", name="pp")
                nc.tensor.matmul(pp[:], lhsT=tsb[:], rhs=wpt[:],
                                 start=True, stop=True)
                ps = spool.tile([P, 2], f32, tag="ps")
                nc.vector.tensor_tensor(out=ps[:], in0=pp[:], in1=bpt[:],
                                        op=AL.add)
                lrelu001(ps[:], "pst")
                nc.sync.dma_start(out=tp_out[gi * P:(gi + 1) * P, :], in_=ps[:])

            wpt = cpool.tile([hd, 2], f32)
            nc.sync.dma_start(out=wpt[:], in_=Wp[:])
            bpt = cpool.tile([P, 2], f32)
            nc.sync.dma_start(out=bpt[:], in_=bpb[:])
            wys = cpool.tile([hd, hdim], f32)
            nc.sync.dma_start(out=wys[:], in_=WyS[:])
            bys = cpool.tile([P, hdim], f32)
            nc.sync.dma_start(out=bys[:], in_=bySb[:])
            wy1 = cpool.tile([hdim, 1], f32)
            nc.sync.dma_start(out=wy1[:], in_=Wy1[:])
            wy0 = cpool.tile([hdim, 1], f32)
            nc.sync.dma_start(out=wy0[:], in_=Wy0[:])
            by1 = cpool.tile([P, 1], f32)
            nc.sync.dma_start(out=by1[:], in_=by1b[:])
            by0 = cpool.tile([P, 1], f32)
            nc.sync.dma_start(out=by0[:], in_=by0b[:])

            def head(key, src_dram, wy, byt):
                hix = rpool.tile([P, tcap // 16], i16, tag=f"hix{key}", name=f"hix_{key}")
                nc.sync.dma_start(out=hix[:], in_=hidx[key][:])
                nt = tcap // P
                g_t = gpool.tile([P, nt, hd], f32, tag="g")
                nc.gpsimd.dma_gather(g_t[:], src_dram[:], hix[:], tcap, tcap,
                                     hd, queue_num=next_q())
                yt = spool.tile([P, nt], f32, tag=f"y{key}")
                for t in range(nt):
                    tpp = psum.tile([P, P], f32, space="PSUM", tag="tp", name="tpp")
                    nc.tensor.transpose(out=tpp[:], in_=g_t[:, t, :],
                                        identity=ident[:])
                    tsb = wpool.tile([P, P], f32, tag="tsb")
                    nc.vector.tensor_copy(out=tsb[:], in_=tpp[:])
                    m1 = psum.tile([P, hdim], f32, space="PSUM", tag="mm", name="m1")
                    nc.tensor.matmul(m1[:], lhsT=tsb[:], rhs=wys[:],
                                     start=True, stop=True)
                    h1 = wpool.tile([P, hdim], f32, tag="h1")
                    nc.vector.tensor_tensor(out=h1[:], in0=m1[:], in1=bys[:],
                                            op=AL.add)
                    lrelu001(h1[:], "h1t")
                    t2 = psum.tile([hdim, P], f32, space="PSUM", tag="tp", name="t2")
                    nc.tensor.transpose(out=t2[:], in_=h1[:],
                                        identity=ident[:])
                    t2s = wpool.tile([hdim, P], f32, tag="t2s")
                    nc.vector.tensor_copy(out=t2s[:], in_=t2[:])
                    m2 = psum.tile([P, 1], f32, space="PSUM", tag="mm", name="m2")
                    nc.tensor.matmul(m2[:], lhsT=t2s[:], rhs=wy[:],
                                     start=True, stop=True)
                    y1t = spool.tile([P, 1], f32, tag="y1t")
                    nc.vector.tensor_tensor(out=y1t[:], in0=m2[:], in1=byt[:],
                                            op=AL.add)
                    lrelu001(y1t[:], "y1tt")
                    nc.vector.tensor_copy(out=yt[:, t:t + 1], in_=y1t[:])
                nc.sync.dma_start(out=y_out[key][:], in_=yt[:])

            # ---------------- orchestration ----------------
            build_table("r", 1)
            allgather("r", 1)
            build_table("f", 1)
            allgather("f", 1)
            conv("r", 1)
            build_table("r", 2)
            allgather("r", 2)
            conv("f", 1)
            build_table("f", 2)
            allgather("f", 2)
            conv("r", 2)
            conv("f", 2)
            head("tr", xz2_out, wy1, by1)
            head("cr", xz2_out, wy0, by0)
            head("tf", xfz2_out, wy0, by0)
            head("cf", xfz2_out, wy1, by1)

    nc.compile()
    return nc


# ----------------------------------------------------------------------------
# Entry point
# ----------------------------------------------------------------------------

def kernel(x, edge_index, fake_x, fake_edge_index, treat_idx, control_idx,
           W1, a_src1, a_dst1, b1, W2, a_src2, a_dst2, b2,
           WyS, byS, Wy1, by1, Wy0, by0, Wp, bp,
           _half_rows=32768, _debug=False, _trace=False):
    sys.path.insert(0, "/opt/trn_rl_repo")
    from concourse.bass_utils import run_bass_kernel_spmd

    x = np.asarray(x, np.float32)
    fake_x = np.asarray(fake_x, np.float32)
    edge_index = np.asarray(edge_index)
    fake_edge_index = np.asarray(fake_edge_index)
    treat_idx = np.asarray(treat_idx).astype(np.int64)
    control_idx = np.asarray(control_idx).astype(np.int64)

    N, in_dim = x.shape
    hd = np.asarray(W1).shape[1]
    hdim = hd // HEADS
    local = N // NCORES
    groups = int(np.ceil(local / P))
    shard = (groups + 1) * P
    half_rows = _half_rows

    row_r, idx_r, instrs_r = prep_graph(edge_index, N, local, groups, shard,
                                        half_rows)
    row_f, idx_f, instrs_f = prep_graph(fake_edge_index, N, local, groups,
                                        shard, half_rows)

    # head lists
    tr_rows, tr_core, tr_slot, cap1 = _head_lists(treat_idx, row_r, local, shard)
    cr_rows, cr_core, cr_slot, cap2 = _head_lists(control_idx, row_r, local, shard)
    tf_rows, tf_core, tf_slot, cap3 = _head_lists(treat_idx, row_f, local, shard)
    cf_rows, cf_core, cf_slot, cap4 = _head_lists(control_idx, row_f, local, shard)
    tcap = max(cap1, cap2, cap3, cap4, P)
    def padrows(r, cap):
        out = np.zeros((NCORES, tcap), np.int64)
        out[:, :r.shape[1]] = r
        return out
    tr_rows, cr_rows, tf_rows, cf_rows = (padrows(r, tcap) for r in
                                          (tr_rows, cr_rows, tf_rows, cf_rows))

    meta_key = (N, in_dim, hd, half_rows, tcap, _debug,
                tuple((h, tuple(c)) for h, c in instrs_r),
                tuple((h, tuple(c)) for h, c in instrs_f))
    import hashlib
    kh = hashlib.sha256(repr(meta_key).encode()).hexdigest()
    if kh not in _PROGRAM_CACHE:
        cfg = dict(groups=groups, shard=shard, half_rows=half_rows,
                   in_dim=in_dim, hd=hd, instrs_r=instrs_r, instrs_f=instrs_f,
                   tcap=tcap, debug=_debug)
        _PROGRAM_CACHE[kh] = build_program(cfg)
    nc = _PROGRAM_CACHE[kh]

    Wext1 = _fold_w(np.asarray(W1, np.float32), np.asarray(a_src1, np.float32),
                    np.asarray(a_dst1, np.float32))
    Wext2 = _fold_w(np.asarray(W2, np.float32), np.asarray(a_src2, np.float32),
                    np.asarray(a_dst2, np.float32))

    # per-core inputs
    pos_r = row_r - (np.arange(N) // local) * shard  # local positions
    pos_f = row_f - (np.arange(N) // local) * shard
    in_maps = []
    bb = lambda v, w: np.broadcast_to(np.asarray(v, np.float32), (P, w)).copy()
    for c in range(NCORES):
        nodes = np.arange(c * local, (c + 1) * local)
        xTr = np.zeros((P, groups * P), np.float32)
        xTr[:, pos_r[nodes]] = x[nodes].T
        xTf = np.zeros((P, groups * P), np.float32)
        xTf[:, pos_f[nodes]] = fake_x[nodes].T
        m = {
            "xT_r": xTr, "xT_f": xTf,
            "idx_r": idx_r[c], "idx_f": idx_f[c],
            "Wext1": Wext1, "Wext2": Wext2,
            "b1b": bb(b1, hd), "b2b": bb(b2, hd),
            "WyS": np.asarray(WyS, np.float32),
            "bySb": bb(byS, hdim),
            "Wy1": np.asarray(Wy1, np.float32),
            "Wy0": np.asarray(Wy0, np.float32),
            "by1b": bb(by1, 1), "by0b": bb(by0, 1),
            "Wp": np.asarray(Wp, np.float32), "bpb": bb(bp, 2),
            "hidx_tr": _idx16(tr_rows[c][None])[0],
            "hidx_cr": _idx16(cr_rows[c][None])[0],
            "hidx_tf": _idx16(tf_rows[c][None])[0],
            "hidx_cf": _idx16(cf_rows[c][None])[0],
        }
        in_maps.append(m)

    res = run_bass_kernel_spmd(nc, in_maps, list(range(NCORES)), trace=_trace)
    if _trace:
        kernel._last_exec_ns = res.exec_time_ns
    r = res.results

    # reassemble outputs
    xZ2 = np.zeros((N, hd), np.float32)
    xfZ2 = np.zeros((N, hd), np.float32)
    tprob = np.zeros((N, 2), np.float32)
    for c in range(NCORES):
        nodes = np.arange(c * local, (c + 1) * local)
        xZ2[nodes] = r[c]["xz2"][pos_r[nodes]]
        xfZ2[nodes] = r[c]["xfz2"][pos_f[nodes]]
        tprob[nodes] = r[c]["tprob"][pos_r[nodes]]

    def gather_y(key, core, slot):
        out = np.zeros(len(core), np.float32)
        for c in range(NCORES):
            yc = r[c][f"y_{key}"]  # [P, tcap//P]; item i at [i%P, i//P]
            sel = core == c
            s = slot[sel]
            out[sel] = yc[s % P, s // P]
        return out

    y1 = gather_y("tr", tr_core, tr_slot)
    y0 = gather_y("cr", cr_core, cr_slot)
    yc0 = gather_y("tf", tf_core, tf_slot)
    yc1 = gather_y("cf", cf_core, cf_slot)

    out = (y1, yc0, y0, yc1, xZ2, xfZ2, tprob)
    if _debug:
        dbg = {}
        xZ1 = np.zeros((N, hd), np.float32)
        for c in range(NCORES):
            nodes = np.arange(c * local, (c + 1) * local)
            xZ1[nodes] = r[c]["xz1"][pos_r[nodes]]
        dbg["xZ1"] = xZ1
        return out, dbg
    return out
